# Initial kernel scaffold
#
"""Multi-head attention (RoPE on k/v) Bass kernel for 8 TRN2 NeuronCores.

Sharding: tensor-parallel over heads (2 heads/core, both batches) for the
QKV projections + attention; one AllToAll redistributes context to a
row-sharded output projection. All matmuls in float32r (TF32-like).

Self-contained: hardcodes shapes from the problem spec.
"""
import os
import sys
import types

import numpy as np


def _install_ntff_hook():
    """antenv.axon_hooks is missing from this image; synthesize it so
    run_bass_kernel_spmd(trace=True) works. Harmless when trace=False."""
    if "antenv.axon_hooks" in sys.modules:
        return
    try:
        from trn_agent_boot.trn_boot import _ntff_profile_via_ctypes

        hook = _ntff_profile_via_ctypes("/opt/axon/libaxon_pjrt.so")
    except Exception:
        hook = None
    mod = types.ModuleType("antenv.axon_hooks")
    mod._hook = hook
    mod.get_axon_ntff_profile_hook = lambda: mod._hook
    mod.set_axon_ntff_profile_hook = lambda h: setattr(mod, "_hook", h)
    sys.modules["antenv.axon_hooks"] = mod
    try:
        import antenv

        antenv.axon_hooks = mod
    except Exception:
        pass


_install_ntff_hook()

import concourse.bass as bass  # noqa: E402
import concourse.mybir as mybir  # noqa: E402
import concourse.tile as tile  # noqa: E402
from concourse import bacc  # noqa: E402
from concourse.bass import ds  # noqa: E402
from concourse.bass_utils import run_bass_kernel_spmd  # noqa: E402

B, S, D, H = 2, 2048, 1024, 16
R = B * S              # 4096 flattened rows
NC = 8                 # cores
HPC = H // NC          # 2 heads per core
CW = D // NC           # 128 ctx cols per core
DH = D // H            # 64 head dim
RW = R // NC           # 512 output rows per core
F32 = mybir.dt.float32
F32R = mybir.dt.float16  # matmul dtype (was float32r; fp16 = full PE rate)
FP = np.float32


def _round_fp32r(a: np.ndarray) -> np.ndarray:
    """Round fp32 to the fp32r (11 explicit mantissa bits) grid, RNE."""
    u = np.ascontiguousarray(a, dtype=np.float32).view(np.uint32)
    low = u & np.uint32(0xFFF)
    base = u & np.uint32(0xFFFFF000)
    rnd = (low > 0x800) | ((low == 0x800) & (((u >> 12) & 1) == 1))
    return (base + (rnd.astype(np.uint32) << 12)).view(np.float32)


def _perm_local() -> np.ndarray:
    """Within a core's 128-col slice: [A-evens, A-odds, B-evens, B-odds]."""
    a_ev = np.arange(0, 64, 2)
    a_od = np.arange(1, 64, 2)
    return np.concatenate([a_ev, a_od, 64 + a_ev, 64 + a_od])


def _build_program():
    nc = bacc.Bacc("TRN2", target_bir_lowering=False, debug=False, num_devices=NC)

    # ---- external I/O ----
    xq_d = nc.dram_tensor("xq", [D, R], F32R, kind="ExternalInput").ap()
    xk_d = nc.dram_tensor("xk", [D, R], F32R, kind="ExternalInput").ap()
    xv_d = nc.dram_tensor("xv", [D, R], F32R, kind="ExternalInput").ap()
    wq_d = nc.dram_tensor("wq", [D, CW], F32R, kind="ExternalInput").ap()
    wk_d = nc.dram_tensor("wk", [D, CW], F32R, kind="ExternalInput").ap()
    wv_d = nc.dram_tensor("wv", [D, CW], F32R, kind="ExternalInput").ap()
    wp_d = nc.dram_tensor("wp", [D, D], F32R, kind="ExternalInput").ap()
    cs_d = nc.dram_tensor("cs", [128, R], F32R, kind="ExternalInput").ap()
    sn_d = nc.dram_tensor("sn", [128, R], F32R, kind="ExternalInput").ap()
    sw_d = nc.dram_tensor("sw", [128, 128], F32R, kind="ExternalInput").ap()
    id_d = nc.dram_tensor("ident", [128, 128], F32R, kind="ExternalInput").ap()
    on_d = nc.dram_tensor("ones", [128, 64], F32R, kind="ExternalInput").ap()
    bi_d = nc.dram_tensor("bias", [128, D], F32, kind="ExternalInput").ap()
    out_d = nc.dram_tensor("out", [RW, D], F32, kind="ExternalOutput").ap()

    KT = D // 128   # 8 contraction tiles for projections
    RT = R // 512   # 8 row tiles
    QT = S // 512   # 4 q tiles per batch
    ST = S // 128   # 16 k tiles per batch

    with tile.TileContext(nc) as tc:
        with (
            tc.tile_pool(name="const", bufs=1) as const,
            tc.tile_pool(name="persist", bufs=1) as persist,
            tc.tile_pool(name="dram", bufs=1, space="DRAM") as dram,
            tc.tile_pool(name="wqkv", bufs=1) as wpool,
            tc.tile_pool(name="trig", bufs=1) as trig,
            tc.tile_pool(name="xin", bufs=3) as xin,
            tc.tile_pool(name="vtmp", bufs=1) as vtmp,
            tc.tile_pool(name="rope", bufs=3) as rp,
            tc.tile_pool(name="epool", bufs=2) as ep,
            tc.tile_pool(name="norm", bufs=1) as npl,
            tc.tile_pool(name="oio", bufs=2) as oio,
            tc.tile_pool(name="pp", bufs=1, space="PSUM") as pp,
            tc.tile_pool(name="spsum", bufs=1, space="PSUM") as sp,
            tc.tile_pool(name="cpsum", bufs=1, space="PSUM") as cp,
        ):
            # ---- constants / weights ----
            sw_sb = const.tile([128, 128], F32R)
            nc.sync.dma_start(sw_sb[:], sw_d[:])
            id_sb = const.tile([128, 128], F32R)
            nc.sync.dma_start(id_sb[:], id_d[:])
            on_sb = const.tile([128, 64], F32R)
            nc.sync.dma_start(on_sb[:], on_d[:])
            wk_sb = wpool.tile([128, KT, CW], F32R)
            nc.sync.dma_start(wk_sb[:], wk_d.rearrange("(kt p) m -> p kt m", p=128))
            wv_sb = wpool.tile([128, KT, CW], F32R)
            nc.sync.dma_start(wv_sb[:], wv_d.rearrange("(kt p) m -> p kt m", p=128))
            wq_sb = wpool.tile([128, KT, CW], F32R)
            nc.sync.dma_start(wq_sb[:], wq_d.rearrange("(kt p) m -> p kt m", p=128))
            cs_sb = trig.tile([128, R], F32R)
            sn_sb = trig.tile([128, R], F32R)
            bi_sb = const.tile([128, D], F32)
            wp_sb = const.tile([128, KT, D], F32R)

            qpT = persist.tile([128, R], F32R)
            kpT = persist.tile([128, R], F32R)
            vaug = persist.tile([128, R // 128, 256], F32R)
            nc.vector.tensor_copy(
                vaug[:, :, 64:128],
                on_sb[:, :, None].rearrange("p o n -> p n o").to_broadcast((128, R // 128, 64)),
            )
            nc.vector.tensor_copy(
                vaug[:, :, 192:256],
                on_sb[:, :, None].rearrange("p o n -> p n o").to_broadcast((128, R // 128, 64)),
            )
            vpT = vtmp.tile([128, R], F32R)

            _chunk_rows = [128, 128, 128, 64, 64]
            a2a_ins = [dram.tile([NC, 128, _chunk_rows[p]], F32R,
                                 name=f"a2ain{p}", tag=f"a2ain{p}")
                       for p in range(5)]
            a2a_outs = [dram.tile([NC, 128, _chunk_rows[p]], F32R,
                                  name=f"a2aout{p}", tag=f"a2aout{p}")
                        for p in range(5)]

            RTB = RT // B     # 4 row-tiles per batch
            CHUNKS = [(0, 2), (2, 4), (4, 6), (6, 7), (7, 8)]  # groups per A2A

            def prefetch_x(ti, rt):
                xd = (xk_d, xv_d, xq_d)[ti]
                rsl = ds(rt * 512, 512)
                x_sb = xin.tile([128, KT, 512], F32R, tag="xin", name="x_sb")
                nc.sync.dma_start(
                    x_sb[:],
                    xd.rearrange("(kt p) r -> p kt r", p=128)[:, :, rsl],
                )
                return x_sb

            def emit_proj_tile(ti, rt, x_sb=None):
                xd, wsb, dest, do_rope = (
                    (xk_d, wk_sb, kpT, True),
                    (xv_d, wv_sb, vpT, True),
                    (xq_d, wq_sb, qpT, False),
                )[ti]
                rsl = ds(rt * 512, 512)
                if x_sb is None:
                    x_sb = prefetch_x(ti, rt)
                ps = pp.tile([128, 512], F32, tag="proj", bufs=1, name="ps")
                for kt in range(KT):
                    nc.tensor.matmul(
                        ps[:], wsb[:, kt], x_sb[:, kt],
                        start=(kt == 0), stop=(kt == KT - 1),
                    )
                if not do_rope:
                    nc.vector.tensor_copy(dest[:, rsl], ps[:])
                else:
                    raw = rp.tile([128, 512], F32R, tag="raw", name="raw")
                    nc.vector.tensor_copy(raw[:], ps[:])
                    sps = pp.tile([128, 512], F32, tag="swp", bufs=1, name="sps")
                    nc.tensor.matmul(sps[:], sw_sb[:], raw[:], start=True, stop=True)
                    t1 = rp.tile([128, 512], F32, tag="t1", name="t1")
                    nc.gpsimd.tensor_tensor(
                        t1[:], raw[:], cs_sb[:, rsl], mybir.AluOpType.mult)
                    t2 = rp.tile([128, 512], F32, tag="t2", name="t2")
                    nc.vector.tensor_tensor(
                        t2[:], sps[:], sn_sb[:, rsl], mybir.AluOpType.mult)
                    nc.vector.tensor_tensor(
                        dest[:, rsl], t1[:], t2[:], mybir.AluOpType.add)

            def emit_vtrans(ct):
                tpx = pp.tile([128, 512], F32R, tag="swp", bufs=1, name="tpx")
                tp = tpx[:, 0:128]
                nc.tensor.transpose(tp, vpT[:, ds(ct * 128, 128)], id_sb[:])
                nc.vector.tensor_copy(vaug[:, ct, 0:64], tp[:, 0:64])
                nc.vector.tensor_copy(vaug[:, ct, 128:192], tp[:, 64:128])

            def emit_attn_qt(bb, qt):
                qsl = ds(bb * S + qt * 512, 512)
                EA = ep.tile([128, ST, 512], F32R, tag="EA", name="EA")
                EB = ep.tile([128, ST, 512], F32R, tag="EB", name="EB")
                cA = cp.tile([128, 512], F32, tag="cA", bufs=1, name="cA")
                cB = cp.tile([128, 512], F32, tag="cB", bufs=1, name="cB")

                def ctx_pair(kt):
                    ct = bb * ST + kt
                    nc.tensor.matmul(cA[:], vaug[:, ct, 0:128], EA[:, kt],
                                     start=(kt == 0), stop=(kt == ST - 1))
                    nc.tensor.matmul(cB[:], vaug[:, ct, 128:256], EB[:, kt],
                                     start=(kt == 0), stop=(kt == ST - 1))

                for g in range(ST // 2):
                    psA = sp.tile([128, 2, 512], F32, tag="sA", bufs=1, name="psA")
                    psB = sp.tile([128, 2, 512], F32, tag="sB", bufs=1, name="psB")
                    for j in range(2):
                        kt = g * 2 + j
                        ksl = ds(bb * S + kt * 128, 128)
                        nc.tensor.matmul(
                            psA[:, j], kpT[0:64, ksl], qpT[0:64, qsl],
                            start=True, stop=True)
                        nc.tensor.matmul(
                            psB[:, j], kpT[64:128, ksl], qpT[64:128, qsl],
                            start=True, stop=True)
                    nc.scalar.activation(
                        EA[:, 2 * g:2 * g + 2], psA[:],
                        mybir.ActivationFunctionType.Exp, scale=0.125)
                    nc.scalar.activation(
                        EB[:, 2 * g:2 * g + 2], psB[:],
                        mybir.ActivationFunctionType.Exp, scale=0.125)
                    if g >= 1:
                        ctx_pair(2 * (g - 1))
                        ctx_pair(2 * (g - 1) + 1)
                ctx_pair(ST - 2)
                ctx_pair(ST - 1)
                zhi = npl.tile([128, 1024], F32, tag="zhi", name="zhi")
                nc.vector.tensor_copy(zhi[64:128, 0:512], cA[64:128])
                nc.vector.tensor_copy(zhi[64:128, 512:1024], cB[64:128])
                zlo = npl.tile([64, 1024], F32, tag="zlo", name="zlo")
                nc.sync.dma_start(zlo[:], zhi[64:128, :])
                zr = npl.tile([64, 1024], F32, tag="zr", name="zr")
                nc.vector.reciprocal_approx_fast(zr[:], zlo[:])
                ctxA = npl.tile([64, 512], F32R, tag="ctxA", name="ctxA")
                nc.vector.tensor_tensor(
                    ctxA[:], cA[0:64], zr[:, 0:512], mybir.AluOpType.mult)
                ctxB = npl.tile([64, 512], F32R, tag="ctxB", name="ctxB")
                nc.vector.tensor_tensor(
                    ctxB[:], cB[0:64], zr[:, 512:1024], mybir.AluOpType.mult)
                shard = bb * QT + qt
                ck = next(i for i, (a, b) in enumerate(CHUNKS) if a <= shard < b)
                a, b = CHUNKS[ck]
                rsl2 = ds((shard - a) * 64, 64)
                nc.sync.dma_start(
                    a2a_ins[ck][:, 0:64, rsl2].rearrange("j p r -> p j r"),
                    ctxA[:].rearrange("p (j r) -> p j r", j=NC))
                nc.sync.dma_start(
                    a2a_ins[ck][:, 64:128, rsl2].rearrange("j p r -> p j r"),
                    ctxB[:].rearrange("p (j r) -> p j r", j=NC))
                if shard == b - 1:
                    nc.gpsimd.collective_compute(
                        "AllToAll",
                        mybir.AluOpType.bypass,
                        replica_groups=[list(range(NC))],
                        ins=[a2a_ins[ck].opt()],
                        outs=[a2a_outs[ck].opt()],
                    )
                    nrows = (b - a) * 64
                    lh = oio.tile([128, NC, 128], F32R, tag="lh", name="lh")
                    nc.sync.dma_start(
                        lh[:, :, 0:nrows],
                        a2a_outs[ck][:].rearrange("j p r -> p j r"),
                    )
                    for oc in range(2):
                        po = pp.tile([128, 512], F32, tag="swp", bufs=1, name="po")
                        for j in range(NC):
                            nc.tensor.matmul(
                                po[0:nrows], lh[:, j, 0:nrows],
                                wp_sb[:, j, ds(oc * 512, 512)],
                                start=(j == 0), stop=(j == NC - 1))
                        ob = oio.tile([128, 512], F32, tag="ob", name="ob")
                        nc.vector.tensor_tensor(
                            ob[0:nrows], po[0:nrows], bi_sb[0:nrows, ds(oc * 512, 512)],
                            mybir.AluOpType.add)
                        nc.sync.dma_start(
                            out_d[ds(a * 64, nrows), ds(oc * 512, 512)], ob[0:nrows])

            # ---- batch 0 projections ----
            x00 = prefetch_x(0, 0)
            x01 = prefetch_x(0, 1)
            nc.sync.dma_start(cs_sb[:], cs_d[:])
            nc.sync.dma_start(sn_sb[:], sn_d[:])
            emit_proj_tile(0, 0, x00)
            emit_proj_tile(0, 1, x01)
            for rt in range(2, RTB):
                emit_proj_tile(0, rt)
            for ti in (1, 2):
                for rt in range(0, RTB):
                    emit_proj_tile(ti, rt)
            nc.sync.dma_start(bi_sb[:], bi_d[:])
            nc.sync.dma_start(wp_sb[:], wp_d.rearrange("(kt p) o -> p kt o", p=128))
            for ct in range(0, S // 128):
                emit_vtrans(ct)
            # ---- batch 0 attention; batch 1 projections interleaved ----
            # piece order: k (scores dep), q (scores dep), v (ctx dep)
            b1_pieces = ([(0, rt) for rt in range(RTB, 2 * RTB)]
                         + [(2, rt) for rt in range(RTB, 2 * RTB)]
                         + [(1, rt) for rt in range(RTB, 2 * RTB)])
            for qt in range(QT):
                emit_attn_qt(0, qt)
                for ti, rt in b1_pieces[qt * 3:(qt + 1) * 3]:
                    emit_proj_tile(ti, rt)
            for ct in range(S // 128, 2 * (S // 128)):
                emit_vtrans(ct)
            for qt in range(QT):
                emit_attn_qt(1, qt)

    nc.compile()
    return nc


_PROGRAM = None


def _get_program():
    global _PROGRAM
    if _PROGRAM is None:
        _PROGRAM = _build_program()
    return _PROGRAM


def _host_prep(q, k, v, Wq, Wk, Wv, Wp, bp):
    """Build the 8 per-core input maps."""
    rr = lambda a: np.ascontiguousarray(a, dtype=np.float32).astype(np.float16)
    xqT = rr(q.reshape(R, D).T)
    xkT = rr(k.reshape(R, D).T)
    xvT = rr(v.reshape(R, D).T)

    pl = _perm_local()
    perm_global = np.concatenate([128 * c + pl for c in range(NC)])
    wpT = rr(np.ascontiguousarray(Wp.T[perm_global, :]))

    # trig tables
    half = D // 2
    pos = np.arange(S, dtype=np.float64)
    theta = 1.0 / (10000.0 ** (2.0 * np.arange(half, dtype=np.float64) / D))
    ang = pos[:, None] * theta[None, :]          # [S, half]
    cosf = np.cos(ang).astype(FP)                # [S, half]
    sinf = np.sin(ang).astype(FP)

    sw = np.zeros((128, 128), np.float16)
    for m in range(128):
        p = (m + 32) % 64 + 64 * (m // 64)
        sw[p, m] = 1.0
    ident = np.eye(128, dtype=np.float16)
    ones = np.ones((128, 64), np.float16)
    bias = np.broadcast_to(bp.astype(FP), (128, D)).copy()

    in_maps = []
    for c in range(NC):
        cols = 128 * c + pl
        wq_c = rr(np.ascontiguousarray(Wq[cols, :].T))
        wk_c = rr(np.ascontiguousarray(Wk[cols, :].T))
        wv_c = rr(np.ascontiguousarray(Wv[cols, :].T))
        # pair index per partition p (see _perm_local ordering)
        j = np.empty(128, np.int64)
        j[0:32] = 64 * c + np.arange(32)
        j[32:64] = 64 * c + np.arange(32)
        j[64:96] = 64 * c + 32 + np.arange(32)
        j[96:128] = 64 * c + 32 + np.arange(32)
        cs1 = cosf[:, j].T                        # [128, S]
        sn1 = sinf[:, j].T.copy()
        sn1[0:32] *= -1.0
        sn1[64:96] *= -1.0
        cs = np.tile(cs1, (1, B)).astype(np.float16)      # [128, R]
        sn = np.tile(sn1, (1, B)).astype(np.float16)
        in_maps.append({
            "xq": xqT, "xk": xkT, "xv": xvT,
            "wq": wq_c, "wk": wk_c, "wv": wv_c,
            "wp": wpT, "cs": cs, "sn": sn,
            "sw": sw, "ident": ident, "ones": ones, "bias": bias,
        })
    return in_maps


def run(inputs, trace=False, trace_cores=None):
    nc = _get_program()
    in_maps = _host_prep(**inputs)
    res = run_bass_kernel_spmd(
        nc, in_maps, core_ids=list(range(NC)), trace=trace,
        trace_cores=trace_cores,
    )
    outs = np.stack([res.results[c]["out"] for c in range(NC)])  # [c, 512, D]
    # local row (128p + 64g' + i) on core c == global row 512*(2p+g') + 64c + i
    full = np.empty((NC, NC, 64, D), np.float32)  # [group(2p+g'), core, i, D]
    lo = outs.reshape(NC, NC, 64, D)              # [core, (2p,g'), i, D]
    full = lo.transpose(1, 0, 2, 3).reshape(B, S, D)
    return full, res


def kernel(**inputs) -> np.ndarray:
    trace = bool(int(os.environ.get("TRN_TRACE", "0")))
    full, res = run(inputs, trace=trace)
    if trace and res.exec_time_ns is not None:
        print(f"HW exec time: {res.exec_time_ns} ns")
    return full



# revision 41
# speedup vs baseline: 1.2000x; 1.2000x over previous
"""Multi-head attention (RoPE on k/v) Bass kernel for 8 TRN2 NeuronCores.

Sharding: tensor-parallel over heads (2 heads/core, both batches) for the
QKV projections + attention; AllToAlls redistribute context to a
row-sharded output projection. Matmuls in fp16.

Schedule: PE warm-up during input DMA, pre-sync collective to absorb CC
init/skew, k/v/q(b0) projections up front, then 8 attention steps with
scores staged through SBUF for big-batch exp, ctx matmuls software-
pipelined across steps, and b1-projection / output-projection pieces
slotted into the tensor stalls.

Self-contained: hardcodes shapes from the problem spec.
"""
import os
import sys
import types

import numpy as np


def _install_ntff_hook():
    """antenv.axon_hooks is missing from this image; synthesize it so
    run_bass_kernel_spmd(trace=True) works. Harmless when trace=False."""
    if "antenv.axon_hooks" in sys.modules:
        return
    try:
        from trn_agent_boot.trn_boot import _ntff_profile_via_ctypes

        hook = _ntff_profile_via_ctypes("/opt/axon/libaxon_pjrt.so")
    except Exception:
        hook = None
    mod = types.ModuleType("antenv.axon_hooks")
    mod._hook = hook
    mod.get_axon_ntff_profile_hook = lambda: mod._hook
    mod.set_axon_ntff_profile_hook = lambda h: setattr(mod, "_hook", h)
    sys.modules["antenv.axon_hooks"] = mod
    try:
        import antenv

        antenv.axon_hooks = mod
    except Exception:
        pass


_install_ntff_hook()

import concourse.bass as bass  # noqa: E402
import concourse.mybir as mybir  # noqa: E402
import concourse.tile as tile  # noqa: E402
from concourse import bacc  # noqa: E402
from concourse.bass import ds  # noqa: E402
from concourse.bass_utils import run_bass_kernel_spmd  # noqa: E402

B, S, D, H = 2, 2048, 1024, 16
R = B * S              # 4096 flattened rows
NC = 8                 # cores
HPC = H // NC          # 2 heads per core
CW = D // NC           # 128 ctx cols per core
DH = D // H            # 64 head dim
RW = R // NC           # 512 output rows per core
F32 = mybir.dt.float32
F32R = mybir.dt.float16  # matmul dtype
FP = np.float32

KT = D // 128   # 8 contraction tiles for projections
RT = R // 512   # 8 row tiles (b0: 0-3, b1: 4-7)
QT = S // 512   # 4 q tiles per batch
ST = S // 128   # 16 k tiles per batch
CHUNKS = [(0, 2), (2, 4), (4, 6), (6, 8)]  # shard groups per A2A


def _perm_local() -> np.ndarray:
    """Within a core's 128-col slice: [A-evens, A-odds, B-evens, B-odds]."""
    a_ev = np.arange(0, 64, 2)
    a_od = np.arange(1, 64, 2)
    return np.concatenate([a_ev, a_od, 64 + a_ev, 64 + a_od])


def _build_program():
    nc = bacc.Bacc("TRN2", target_bir_lowering=False, debug=False, num_devices=NC)

    # ---- external I/O (host-retiled for contiguous per-partition DMA) ----
    xq_d = nc.dram_tensor("xq", [128, RT, KT, 512], F32R, kind="ExternalInput").ap()
    xk_d = nc.dram_tensor("xk", [128, RT, KT, 512], F32R, kind="ExternalInput").ap()
    xv_d = nc.dram_tensor("xv", [128, RT, KT, 512], F32R, kind="ExternalInput").ap()
    wq_d = nc.dram_tensor("wq", [128, KT, CW], F32R, kind="ExternalInput").ap()
    wk_d = nc.dram_tensor("wk", [128, KT, CW], F32R, kind="ExternalInput").ap()
    wv_d = nc.dram_tensor("wv", [128, KT, CW], F32R, kind="ExternalInput").ap()
    wp_d = nc.dram_tensor("wp", [128, KT, D], F32R, kind="ExternalInput").ap()
    cs_d = nc.dram_tensor("cs", [128, R], F32R, kind="ExternalInput").ap()
    sn_d = nc.dram_tensor("sn", [128, R], F32R, kind="ExternalInput").ap()
    sw_d = nc.dram_tensor("sw", [128, 128], F32R, kind="ExternalInput").ap()
    id_d = nc.dram_tensor("ident", [128, 128], F32R, kind="ExternalInput").ap()
    on_d = nc.dram_tensor("ones", [128, 64], F32R, kind="ExternalInput").ap()
    bi_d = nc.dram_tensor("bias", [128, D], F32, kind="ExternalInput").ap()
    out_d = nc.dram_tensor("out", [RW, D], F32, kind="ExternalOutput").ap()

    with tile.TileContext(nc) as tc:
        with (
            tc.tile_pool(name="const", bufs=1) as const,
            tc.tile_pool(name="persist", bufs=1) as persist,
            tc.tile_pool(name="dram", bufs=1, space="DRAM") as dram,
            tc.tile_pool(name="wqkv", bufs=1) as wpool,
            tc.tile_pool(name="trig", bufs=1) as trig,
            tc.tile_pool(name="xin", bufs=3) as xin,
            tc.tile_pool(name="vtmp", bufs=1) as vtmp,
            tc.tile_pool(name="rope", bufs=3) as rp,
            tc.tile_pool(name="epool", bufs=2) as ep,
            tc.tile_pool(name="norm", bufs=1) as npl,
            tc.tile_pool(name="oio", bufs=2) as oio,
            tc.tile_pool(name="pp", bufs=1, space="PSUM") as pp,
            tc.tile_pool(name="spsum", bufs=1, space="PSUM") as sp,
            tc.tile_pool(name="cpsum", bufs=1, space="PSUM") as cp,
        ):
            # ---- phase 0: tiny consts, PE warm-up, pre-sync ----
            sw_sb = const.tile([128, 128], F32R)
            nc.sync.dma_start(sw_sb[:], sw_d[:])

            # warm the PE (HAM clock-gate) with throwaway matmuls while
            # inputs stream in; ~40 * 107ns cold = ~4.3us of busy.
            warm_ps = pp.tile([128, 512], F32, tag="swp", bufs=1, name="warm")
            for _ in range(72):
                nc.tensor.matmul(warm_ps[:, 0:128], sw_sb[:], sw_sb[:],
                                 start=True, stop=True)
            warm_out = const.tile([128, 4], F32, name="warm_out")
            nc.vector.tensor_copy(warm_out[:], warm_ps[:, 0:4])
            # preload the exp activation table during the head phase
            nc.scalar.activation(warm_out[:], warm_out[:],
                                 mybir.ActivationFunctionType.Exp, scale=0.0)

            # pre-sync collective: absorbs CC init + inter-core skew during
            # the input-DMA phase instead of at the first real AllToAll.
            pre_in = dram.tile([NC, 1, 16], F32R, name="prein", tag="prein")
            pre_out = dram.tile([NC, 1, 16], F32R, name="preout", tag="preout")
            nc.sync.dma_start(pre_in[:, 0], sw_sb[0:NC, 0:16])
            nc.gpsimd.collective_compute(
                "AllToAll", mybir.AluOpType.bypass,
                replica_groups=[list(range(NC))],
                ins=[pre_in.opt()], outs=[pre_out.opt()],
            )

            id_sb = const.tile([128, 128], F32R)
            nc.sync.dma_start(id_sb[:], id_d[:])
            on_sb = const.tile([128, 64], F32R)
            nc.sync.dma_start(on_sb[:], on_d[:])
            cs_sb = trig.tile([128, R], F32R)
            sn_sb = trig.tile([128, R], F32R)
            wk_sb = wpool.tile([128, KT, CW], F32R)
            nc.sync.dma_start(wk_sb[:], wk_d[:])
            wv_sb = wpool.tile([128, KT, CW], F32R)
            nc.sync.dma_start(wv_sb[:], wv_d[:])
            wq_sb = wpool.tile([128, KT, CW], F32R)
            nc.sync.dma_start(wq_sb[:], wq_d[:])
            bi_sb = const.tile([128, D], F32)
            wp_sb = const.tile([128, KT, D], F32R)

            qpT = persist.tile([128, R], F32R)
            kpT = persist.tile([128, R], F32R)
            vaug = persist.tile([128, R // 128, 256], F32R)
            nc.vector.tensor_copy(
                vaug[:, :, 64:128],
                on_sb[:, :, None].rearrange("p o n -> p n o").to_broadcast((128, R // 128, 64)),
            )
            nc.vector.tensor_copy(
                vaug[:, :, 192:256],
                on_sb[:, :, None].rearrange("p o n -> p n o").to_broadcast((128, R // 128, 64)),
            )
            vpT = vtmp.tile([128, R], F32R)

            a2a_ins = [dram.tile([NC, 128, 128], F32R,
                                 name=f"a2ain{p}", tag=f"a2ain{p}")
                       for p in range(4)]
            a2a_outs = [dram.tile([NC, 128, 128], F32R,
                                  name=f"a2aout{p}", tag=f"a2aout{p}")
                        for p in range(4)]

            # ---------------- building blocks ----------------
            def emit_proj_tile(ti, rt):
                """ti: 0=k (rope), 1=v (rope), 2=q. rt: global row tile."""
                xd, wsb, dest, do_rope = (
                    (xk_d, wk_sb, kpT, True),
                    (xv_d, wv_sb, vpT, True),
                    (xq_d, wq_sb, qpT, False),
                )[ti]
                rsl = ds(rt * 512, 512)
                x_sb = xin.tile([128, KT, 512], F32R, tag="xin", name="x_sb")
                nc.sync.dma_start(x_sb[:], xd[:, rt])
                ps = pp.tile([128, 512], F32, tag="proj", bufs=1, name="ps")
                for kt in range(KT):
                    nc.tensor.matmul(
                        ps[:], wsb[:, kt], x_sb[:, kt],
                        start=(kt == 0), stop=(kt == KT - 1),
                    )
                if not do_rope:
                    nc.vector.tensor_copy(dest[:, rsl], ps[:])
                else:
                    # NOTE: all elementwise on DVE — gpsimd runs ONLY the
                    # collectives (its in-order queue would otherwise stall
                    # rope behind a peer-waiting AllToAll).
                    raw = rp.tile([128, 512], F32R, tag="raw", name="raw")
                    nc.vector.tensor_copy(raw[:], ps[:])
                    sps = pp.tile([128, 512], F32, tag="swp", bufs=1, name="sps")
                    nc.tensor.matmul(sps[:], sw_sb[:], raw[:], start=True, stop=True)
                    t1 = rp.tile([128, 512], F32, tag="t1", name="t1")
                    nc.vector.tensor_tensor(
                        t1[:], raw[:], cs_sb[:, rsl], mybir.AluOpType.mult)
                    t2 = rp.tile([128, 512], F32, tag="t2", name="t2")
                    nc.vector.tensor_tensor(
                        t2[:], sps[:], sn_sb[:, rsl], mybir.AluOpType.mult)
                    nc.vector.tensor_tensor(
                        dest[:, rsl], t1[:], t2[:], mybir.AluOpType.add)

            def emit_vtrans(ct):
                tpx = pp.tile([128, 512], F32R, tag="swp", bufs=1, name="tpx")
                tp = tpx[:, 0:128]
                nc.tensor.transpose(tp, vpT[:, ds(ct * 128, 128)], id_sb[:])
                nc.vector.tensor_copy(
                    vaug[:, ct].rearrange("p (two s) -> p two s", two=2)[:, :, 0:64],
                    tp.rearrange("p (two s) -> p two s", two=2),
                )

            # per-shard state carried across steps
            shard_state = {}

            def emit_scores(s, fills=(), defer_ctxA=False):
                """scores + exp (PSUM-direct, N=1024/call) for shard s,
                with this shard's head-A ctx pairs and the carried-over
                head-B ctx of shard s-1 (fills) woven into the tensor
                stream so the PE tracks the scalar-paced exp flow."""
                fills = list(fills)
                bb, qt = divmod(s, QT)
                qsl = ds(bb * S + qt * 512, 512)
                EA = ep.tile([128, ST, 512], F32R, tag="EA", name="EA")
                EB = ep.tile([128, ST, 512], F32R, tag="EB", name="EB")
                shard_state[s] = (EA, EB)
                for g in range(ST // 2):
                    psA = sp.tile([128, 2, 512], F32, tag="sA", bufs=1, name="psA")
                    psB = sp.tile([128, 2, 512], F32, tag="sB", bufs=1, name="psB")
                    for j in range(2):
                        kt = g * 2 + j
                        ksl = ds(bb * S + kt * 128, 128)
                        nc.tensor.matmul(
                            psA[:, j], kpT[0:64, ksl], qpT[0:64, qsl],
                            start=True, stop=True)
                    nc.scalar.activation(
                        EA[:, 2 * g:2 * g + 2], psA[:],
                        mybir.ActivationFunctionType.Exp, scale=0.125)
                    for j in range(2):
                        kt = g * 2 + j
                        ksl = ds(bb * S + kt * 128, 128)
                        nc.tensor.matmul(
                            psB[:, j], kpT[64:128, ksl], qpT[64:128, qsl],
                            start=True, stop=True)
                    nc.scalar.activation(
                        EB[:, 2 * g:2 * g + 2], psB[:],
                        mybir.ActivationFunctionType.Exp, scale=0.125)
                    if g >= 1 and not defer_ctxA:
                        emit_ctx(s, 0, 2 * (g - 1), 2 * g)
                    for _ in range(2 if g < 7 else 4):
                        if fills:
                            fills.pop(0)()
                for f in fills:
                    f()

            cpsums = {}

            def get_cpsum(s, h):
                if (s, h) not in cpsums:
                    tag = ("cA", "cB")[h]
                    cpsums[(s, h)] = cp.tile(
                        [128, 512], F32, tag=tag, bufs=1, name=tag)
                return cpsums[(s, h)]

            def emit_ctx_mm(s, h, kt):
                bb = s // QT
                E = shard_state[s][h]
                c = get_cpsum(s, h)
                ct = bb * ST + kt
                nc.tensor.matmul(
                    c[:], vaug[:, ct, ds(128 * h, 128)], E[:, kt],
                    start=(kt == 0), stop=(kt == ST - 1))

            def emit_ctx(s, h, kt0, kt1):
                for kt in range(kt0, kt1):
                    emit_ctx_mm(s, h, kt)

            def emit_drain(s, h):
                """normalize ctx for (shard s, head h) and write a2a input."""
                c = get_cpsum(s, h)
                nm = ("A", "B")[h]
                zhi = npl.tile([128, 512], F32, tag="zhi", name="zhi")
                nc.vector.tensor_copy(zhi[64:128], c[64:128])
                zlo = npl.tile([64, 512], F32, tag=f"zlo{nm}", name=f"zlo{nm}")
                nc.sync.dma_start(zlo[:], zhi[64:128])
                zr = npl.tile([64, 512], F32, tag=f"zr{nm}", name=f"zr{nm}")
                nc.vector.reciprocal_approx_fast(zr[:], zlo[:])
                ctxh = npl.tile([64, 512], F32R, tag=f"ctx{nm}", name=f"ctx{nm}")
                nc.vector.tensor_tensor(
                    ctxh[:], c[0:64], zr[:], mybir.AluOpType.mult)
                ck = next(i for i, (a, b) in enumerate(CHUNKS) if a <= s < b)
                a, b = CHUNKS[ck]
                rsl2 = ds((s - a) * 64, 64)
                dst = a2a_ins[ck][:, ds(64 * h, 64), rsl2].rearrange(
                    "j p r -> p j r")
                nc.sync.dma_start(dst, ctxh[:].rearrange("p (j r) -> p j r", j=NC))

            def emit_a2a(ck):
                nc.gpsimd.collective_compute(
                    "AllToAll", mybir.AluOpType.bypass,
                    replica_groups=[list(range(NC))],
                    ins=[a2a_ins[ck].opt()], outs=[a2a_outs[ck].opt()],
                )

            def emit_outproj(ck, oc):
                """one 512-col half of the output projection for chunk ck."""
                a, b = CHUNKS[ck]
                nrows = (b - a) * 64
                if oc == 0:
                    lh = oio.tile([128, NC, 128], F32R, tag="lh", name="lh")
                    emit_outproj.lh = lh
                    nc.sync.dma_start(
                        lh[:, :, 0:nrows],
                        a2a_outs[ck][:].rearrange("j p r -> p j r"),
                    )
                lh = emit_outproj.lh
                po = pp.tile([128, 512], F32, tag="swp", bufs=1, name="po")
                for j in range(NC):
                    nc.tensor.matmul(
                        po[0:nrows], lh[:, j, 0:nrows],
                        wp_sb[:, j, ds(oc * 512, 512)],
                        start=(j == 0), stop=(j == NC - 1))
                ob = oio.tile([128, 512], F32, tag="ob", name="ob")
                nc.vector.tensor_tensor(
                    ob[0:nrows], po[0:nrows], bi_sb[0:nrows, ds(oc * 512, 512)],
                    mybir.AluOpType.add)
                nc.sync.dma_start(
                    out_d[ds(a * 64, nrows), ds(oc * 512, 512)], ob[0:nrows])

            # ---------------- head phase ----------------
            # only what scores(0) g0-3 need up front: k r0 + q r0. The rest
            # of the b0 projections ride as step-0 fills, paced with the
            # scores loop so the first attention step starts ~25us earlier.
            emit_proj_tile(0, 0)                # k b0 r0 (rope)
            emit_proj_tile(2, 0)                # q b0 qt0
            # trig tables land after the first x tiles: the k-r0 rope only
            # needs them once its projection matmuls finish
            nc.sync.dma_start(cs_sb[:], cs_d[:])
            nc.sync.dma_start(sn_sb[:], sn_d[:])

            # ---------------- attention steps ----------------
            # pieces per step: emitted into the tensor stalls between ctx
            # groups. (ti, rt) proj / ("vt", ct0) vtrans / ("op", ck, oc).
            # NOTE: an ("op", ck, _) piece must sit at least one step after
            # emit_a2a(ck) in program order — pieces are emitted inside
            # emit_scores, BEFORE that step's own a2a issue.
            piece_sched = [
                [(0, 1), (1, 0), (0, 2), ("vt", 0), (0, 3), (1, 1),
                 ("vt", 4), (2, 1), (1, 2), ("vt", 8), (1, 3), ("vt", 12)],
                [(2, 2), (0, 4), (1, 4), ("vt", 16), (0, 5)],
                [(2, 3), (1, 5), ("vt", 20), (0, 6)],
                [(1, 6), ("vt", 24), (0, 7), (1, 7), ("vt", 28), (2, 4)],
                [(2, 5), ("op", 0, 0), ("op", 0, 1)],
                [(2, 6), ("op", 1, 0), ("op", 1, 1)],
                [(2, 7)],
                [("op", 2, 0), ("op", 2, 1)],
            ]
            # A2A ck issued in the step after its last shard's head-B drain.
            a2a_after_drain = {b - 1: ck for ck, (a, b) in enumerate(CHUNKS)}

            def emit_piece(p):
                if p[0] == "vt":
                    for ct in range(p[1], p[1] + 4):
                        emit_vtrans(ct)
                elif p[0] == "op":
                    emit_outproj(p[1], p[2])
                else:
                    emit_proj_tile(p[0], p[1])

            for s in range(8):
                # fillers: ALL of shard s-1's head-B ctx + this step's
                # pieces, pumped into the scalar-paced scores loop.
                fills = []
                if s > 0:
                    fills += [
                        (lambda ss=s - 1, kt=kt: emit_ctx_mm(ss, 1, kt))
                        for kt in range(0, 16)
                    ]
                fills += [(lambda p=p: emit_piece(p)) for p in piece_sched[s]]
                if s == 0:
                    # step 0: head-A ctx deferred behind the v/vtrans fills
                    fills += [
                        (lambda kt=kt: emit_ctx_mm(0, 0, kt))
                        for kt in range(0, 14)
                    ]
                emit_scores(s, fills, defer_ctxA=(s == 0))
                if s > 0:
                    emit_drain(s - 1, 1)
                    if (s - 1) in a2a_after_drain:
                        emit_a2a(a2a_after_drain[s - 1])
                emit_ctx(s, 0, 14, 16)
                emit_drain(s, 0)
                if s == 0:
                    # needed from step 4 (first outproj) — load late so the
                    # step-0 x-tile DMAs keep priority
                    nc.sync.dma_start(bi_sb[:], bi_d[:])
                    nc.sync.dma_start(wp_sb[:], wp_d[:])
            # start shard 7's head-B ctx before the flush to shorten the tail
            emit_ctx(7, 1, 0, 8)

            # flush: shard 7 head-B tail, its drain, last A2As + outprojs
            emit_ctx(7, 1, 8, 16)
            emit_drain(7, 1)
            emit_a2a(3)
            # keep the PE warm through the A2A peer-wait so the final
            # out-projection runs at full clock
            fl_ps = pp.tile([128, 512], F32, tag="swp", bufs=1, name="flwarm")
            for _ in range(36):
                nc.tensor.matmul(fl_ps[:, 0:128], sw_sb[:], sw_sb[:],
                                 start=True, stop=True)
            fl_out = const.tile([128, 4], F32, name="fl_out")
            nc.vector.tensor_copy(fl_out[:], fl_ps[:, 0:4])
            emit_outproj(3, 0)
            emit_outproj(3, 1)

    nc.compile()
    return nc


_PROGRAM = None


def _get_program():
    global _PROGRAM
    if _PROGRAM is None:
        _PROGRAM = _build_program()
    return _PROGRAM


def _host_prep(q, k, v, Wq, Wk, Wv, Wp, bp):
    """Build the 8 per-core input maps."""
    rr = lambda a: np.ascontiguousarray(a, dtype=np.float32).astype(np.float16)

    def tile_x(xT):  # [D, R] -> [128, RT, KT, 512]
        return np.ascontiguousarray(
            xT.reshape(KT, 128, RT, 512).transpose(1, 2, 0, 3))

    xqT = tile_x(rr(q.reshape(R, D).T))
    xkT = tile_x(rr(k.reshape(R, D).T))
    xvT = tile_x(rr(v.reshape(R, D).T))

    pl = _perm_local()
    perm_global = np.concatenate([128 * c + pl for c in range(NC)])
    wpT = rr(np.ascontiguousarray(Wp.T[perm_global, :]))
    wpT = np.ascontiguousarray(wpT.reshape(KT, 128, D).transpose(1, 0, 2))

    # trig tables
    half = D // 2
    pos = np.arange(S, dtype=np.float64)
    theta = 1.0 / (10000.0 ** (2.0 * np.arange(half, dtype=np.float64) / D))
    ang = pos[:, None] * theta[None, :]          # [S, half]
    cosf = np.cos(ang).astype(FP)                # [S, half]
    sinf = np.sin(ang).astype(FP)

    sw = np.zeros((128, 128), np.float16)
    for m in range(128):
        p = (m + 32) % 64 + 64 * (m // 64)
        sw[p, m] = 1.0
    ident = np.eye(128, dtype=np.float16)
    ones = np.ones((128, 64), np.float16)
    bias = np.broadcast_to(bp.astype(FP), (128, D)).copy()

    in_maps = []
    for c in range(NC):
        cols = 128 * c + pl
        wq_c = rr(np.ascontiguousarray(Wq[cols, :].T))
        wk_c = rr(np.ascontiguousarray(Wk[cols, :].T))
        wv_c = rr(np.ascontiguousarray(Wv[cols, :].T))
        wq_c = np.ascontiguousarray(wq_c.reshape(KT, 128, CW).transpose(1, 0, 2))
        wk_c = np.ascontiguousarray(wk_c.reshape(KT, 128, CW).transpose(1, 0, 2))
        wv_c = np.ascontiguousarray(wv_c.reshape(KT, 128, CW).transpose(1, 0, 2))
        # pair index per partition p (see _perm_local ordering)
        j = np.empty(128, np.int64)
        j[0:32] = 64 * c + np.arange(32)
        j[32:64] = 64 * c + np.arange(32)
        j[64:96] = 64 * c + 32 + np.arange(32)
        j[96:128] = 64 * c + 32 + np.arange(32)
        cs1 = cosf[:, j].T                        # [128, S]
        sn1 = sinf[:, j].T.copy()
        sn1[0:32] *= -1.0
        sn1[64:96] *= -1.0
        cs = np.tile(cs1, (1, B)).astype(np.float16)      # [128, R]
        sn = np.tile(sn1, (1, B)).astype(np.float16)
        in_maps.append({
            "xq": xqT, "xk": xkT, "xv": xvT,
            "wq": wq_c, "wk": wk_c, "wv": wv_c,
            "wp": wpT, "cs": cs, "sn": sn,
            "sw": sw, "ident": ident, "ones": ones, "bias": bias,
        })
    return in_maps


def run(inputs, trace=False, trace_cores=None):
    nc = _get_program()
    in_maps = _host_prep(**inputs)
    res = run_bass_kernel_spmd(
        nc, in_maps, core_ids=list(range(NC)), trace=trace,
        trace_cores=trace_cores,
    )
    outs = np.stack([res.results[c]["out"] for c in range(NC)])  # [c, 512, D]
    # local row (128p + 64g' + i) on core c == global row 512*(2p+g') + 64c + i
    lo = outs.reshape(NC, NC, 64, D)              # [core, (2p,g'), i, D]
    full = lo.transpose(1, 0, 2, 3).reshape(B, S, D)
    return full, res


def kernel(**inputs) -> np.ndarray:
    trace = bool(int(os.environ.get("TRN_TRACE", "0")))
    full, res = run(inputs, trace=trace)
    if trace and res.exec_time_ns is not None:
        print(f"HW exec time: {res.exec_time_ns} ns")
    return full
